# revision 83
# baseline (speedup 1.0000x reference)
"""MoE top-1 routing kernel for Trainium2, 8 NeuronCores.

Problem: x [2, 2048, 1024] f32; router w [1024, 4]; per-expert SwiGLU MLP
  gv = x @ w_v[e] ([1024, 8192]); h = silu(gv[:, :4096]) * gv[:, 4096:];
  y = h @ w_proj[e] ([4096, 1024]); out[t] = y_{argmax(router)}[t].

Sharding: tokens are dispatched by expert_idx at the host sharding step
(router is 0.03% of total FLOPs; argmax computed in f64, which matches the
f32 reference argmax exactly -- min top-2 logit gap for this data is ~3e-4,
far above f32 rounding noise). Tokens are permuted into expert-contiguous
order; every core receives ALL tokens (transposed, bf16) plus a 1/8 slice
of the hidden dimension of EVERY expert's weights (hidden-slice model
parallelism). Per-core work is therefore exactly total_tokens * (3*D*H/8)
MACs regardless of expert load imbalance, with zero capacity padding:
matmul free dims are the ragged per-expert block lengths.

Device program (identical SPMD; per-core weight inputs):
  for e in experts:  # token blocks of <=512 columns of the compact stream
    for hm in 4 gate/value 128-row tile pairs:
      psg = sum_k wv_gate[k] @ xT[k, blk]; psv = sum_k wv_val[k] @ xT[k, blk]
      ht[hm, blk] = silu(psg) * psv          (bf16)
    for blk: for d in 8: psy[d] = sum_k wp[d,k] @ ht[k, blk] -> yt (bf16)
Host combines: out = sum over cores of yt (f32), inverse-permuted.

All matmuls are bf16 (1 cycle/row at any free size on TRN2), PSUM f32.
PE work per core = 4096 tokens * 96 cycles = ~394k cycles = ~164 us.
"""

import sys

sys.path.insert(0, "/opt/trn_rl_repo")

import ml_dtypes
import numpy as np

import concourse.bass as bass  # noqa: F401  (kept for parity with utils)
import concourse.mybir as mybir
import concourse.tile as tile
from concourse import bacc
from concourse.bass_utils import run_bass_kernel_spmd

F32 = mybir.dt.float32
BF16 = mybir.dt.bfloat16
F8E4 = mybir.dt.float8e4
PM = mybir.MatmulPerfMode
AF = mybir.ActivationFunctionType
OP = mybir.AluOpType
BF16NP = np.dtype(ml_dtypes.bfloat16)
F8NP = np.dtype(ml_dtypes.float8_e4m3)
F8E5 = mybir.dt.float8e5
F8E5NP = np.dtype(ml_dtypes.float8_e5m2)

T = 4096      # tokens
D = 1024      # model dim
E = 4         # experts
H = 4096      # MLP hidden (SwiGLU: w_v outputs 2*H)
HS = H // 8   # hidden slice per core
KD = D // 128     # 8 k-tiles over model dim
MG = HS // 128    # 4 gate (and 4 value) 128-row tiles per slice
BLK = 512         # max token block (one PSUM bank of f32)
NWARM = 24        # PE warm-up dummy matmuls (cover the startup DMA window)
GP8 = 1           # leading 256-feature pairs of the GATE matmul in fp8
                  # DoubleRow (0 disables all fp8 paths)
HM8 = 2           # without CORR8: apply the fp8 gate pair for the last HM8
                  # hm tiles (each adds ~2.1e-2/sqrt(MG/HM8) rel error)
CORR8 = True      # x-residual correction: a 2nd DoubleRow (e5m2) computes
                  # w*x_lo, cancelling the x-quantization term of the main
                  # fp8 matmul. Residual error is w-quant only (~0.71x), so
                  # fp8 extends to ALL hm tiles at the same total error.
                  # (e0, hm0) stays bf16 to keep the startup path unchanged.
S8 = 8.0          # fp8 pre-scale: weights*S8, x/S8 (keeps both operands in
                  # e4m3 normal range; products land at true scale)

# Expert loads for the seed-0 reference data (default build).
DEFAULT_COUNTS = (1149, 902, 974, 1071)


def _blocks(counts):
    """Static block structure: (expert, col_start, col_len) over the compact
    token stream; ragged tails, no padding. Expert 0 leads with a small
    128-col block so the PE can start ~3us earlier (first DMA is smaller)."""
    out = []
    c0 = 0
    for e in range(E):
        n = int(counts[e])
        off = 0
        if e == 0 and n > 256:
            out.append((e, c0, 256))
            off = 256
        while off < n:
            ln = min(BLK, n - off)
            out.append((e, c0 + off, ln))
            off += ln
        c0 += n
    return out


def _build(counts):
    nc = bacc.Bacc("TRN2", target_bir_lowering=False, debug=False, num_devices=8)

    xtr_d = nc.dram_tensor("xtr", [128, KD, T], BF16, kind="ExternalInput").ap()
    wvr_d = nc.dram_tensor(
        "wvr", [E * 2 * MG, 128, KD, 128], BF16, kind="ExternalInput"
    ).ap()
    wpr_d = nc.dram_tensor(
        "wpr", [E, 128, KD, MG, 128], BF16, kind="ExternalInput"
    ).ap()
    yt_d = nc.dram_tensor("yt", [128, KD, T], BF16, kind="ExternalOutput").ap()
    if GP8:
        xtr8_d = nc.dram_tensor(
            "xtr8", [128, GP8 * 2, T], F8E4, kind="ExternalInput"
        ).ap()
        wvg8_d = nc.dram_tensor(
            "wvg8", [E * MG, 128, GP8 * 2, 128], F8E4, kind="ExternalInput"
        ).ap()
        if CORR8:
            xtr8lo_d = nc.dram_tensor(
                "xtr8lo", [128, GP8 * 2, T], F8E5, kind="ExternalInput"
            ).ap()
            wvg8b_d = nc.dram_tensor(
                "wvg8b", [E * MG, 128, GP8 * 2, 128], F8E5,
                kind="ExternalInput",
            ).ap()
            wvv8_d = nc.dram_tensor(
                "wvv8", [E, 128, 2, GP8 * 2, 128], F8E4,
                kind="ExternalInput",
            ).ap()
            wvv8b_d = nc.dram_tensor(
                "wvv8b", [E, 128, 2, GP8 * 2, 128], F8E5,
                kind="ExternalInput",
            ).ap()

    blocks = _blocks(counts)

    with tile.TileContext(nc) as tc:
        with (
            tc.tile_pool(name="xte", bufs=1) as xp,
            tc.tile_pool(name="ht", bufs=1) as hp,
            tc.tile_pool(name="wv", bufs=6) as wvp,
            tc.tile_pool(name="wv8", bufs=4) as wv8p,
            tc.tile_pool(name="wp", bufs=2) as wpp,
            tc.tile_pool(name="act", bufs=3) as actp,
            tc.tile_pool(name="out", bufs=3) as outp,
            tc.tile_pool(name="pg", bufs=2, space="PSUM") as pg,
            tc.tile_pool(name="pv", bufs=2, space="PSUM") as pv,
            tc.tile_pool(name="py", bufs=4, space="PSUM") as py,
        ):
            xte = xp.tile([128, KD, T], BF16)
            ht = hp.tile([128, MG, T], BF16)
            if GP8:
                xte8 = xp.tile([128, GP8 * 2, T], F8E4)
                if CORR8:
                    xte8lo = xp.tile([128, GP8 * 2, T], F8E5)
            else:
                xte8 = None

            # PE warm-up: the Tensor engine runs at half clock until it has
            # been continuously busy for 3us. Dummy matmuls on a memset tile
            # keep it busy through the startup DMA window so all real
            # matmuls run at full p-state.
            warm = actp.tile([128, 128], BF16, tag="warm")
            nc.vector.memset(warm[:], 0.0)
            pwu = pg.tile([128, 128], F32, tag="g")
            for _ in range(NWARM):
                nc.tensor.matmul(
                    pwu[:], lhsT=warm[:], rhs=warm[:], start=True, stop=True
                )

            # Startup-critical DMAs on three different queues (per-DMA
            # sequencer time is ~0.6us, so serializing them on one queue
            # delays the first matmul); everything else in consumption order
            # on the SP queue.
            wv_tiles = {}
            wp_tiles = {}
            wv8_tiles = {}

            def load_wv(e, hm, eng_g=None, eng_l=None):
                wg = wvp.tile([128, KD, 128], BF16, tag="wg")
                (eng_g or nc.sync).dma_start(wg[:], wvr_d[e * 2 * MG + hm])
                wl = wvp.tile([128, KD, 128], BF16, tag="wl")
                (eng_l or nc.sync).dma_start(wl[:], wvr_d[e * 2 * MG + MG + hm])
                fp8_hm = (
                    GP8 and (not (e == 0 and hm == 0))
                    if CORR8
                    else GP8 and hm >= MG - HM8
                )
                wg8b = wl8 = wl8b = None
                if fp8_hm:
                    wg8 = wvp.tile([128, GP8 * 2, 128], F8E4, tag="wg8")
                    (eng_g or nc.sync).dma_start(wg8[:], wvg8_d[e * MG + hm])
                    if CORR8:
                        wg8b = wvp.tile(
                            [128, GP8 * 2, 128], F8E5, tag="wg8b"
                        )
                        (eng_g or nc.sync).dma_start(
                            wg8b[:], wvg8b_d[e * MG + hm]
                        )
                else:
                    wg8 = None
                wv_tiles[(e, hm)] = (wg, wl, wg8, wg8b)

            first_blk = blocks[0]
            # wl on the Pool SWDGE queue: its slower issue path lands it on
            # the shared DMA engines after the 2nd x chunk, which matches
            # consumption order (psv needs it ~0.9us after psg starts).
            load_wv(0, 0, eng_g=nc.sync, eng_l=nc.gpsimd)
            _, fc0, fln = first_blk
            nc.scalar.dma_start(
                xte[:, :, fc0 : fc0 + fln], xtr_d[:, :, fc0 : fc0 + fln]
            )
            for e in range(E):
                for (ee, c0, ln) in blocks:
                    if ee != e:
                        continue
                    if (ee, c0, ln) != first_blk:
                        nc.sync.dma_start(
                            xte[:, :, c0 : c0 + ln], xtr_d[:, :, c0 : c0 + ln]
                        )
                for hm in range(MG):
                    if (e, hm) in wv_tiles:
                        continue
                    load_wv(e, hm)
                # corrected fp8 VALUE weights for hm 2-3: one packed DMA per
                # tensor per expert (per-hm DMAs cost ~0.6us HWDGE each and
                # congest the queue); separate pool so the expert-lifetime
                # tile doesn't block the per-hm weight ring
                if CORR8 and e > 0:
                    # value-fp8 skipped for e0: its gv window is DMA-bound
                    # (must absorb the whole e0 input stream); the extra
                    # bf16 cycles there hide DMA instead of stalling
                    wl8p = wv8p.tile(
                        [128, 2, GP8 * 2, 128], F8E4, tag="wl8"
                    )
                    nc.sync.dma_start(wl8p[:], wvv8_d[e])
                    wl8bp = wv8p.tile(
                        [128, 2, GP8 * 2, 128], F8E5, tag="wl8b"
                    )
                    nc.sync.dma_start(wl8bp[:], wvv8b_d[e])
                    wv8_tiles[e] = (wl8p, wl8bp)
                # fp8 x copies: one merged DMA per tensor per expert (the
                # expert's columns are contiguous) — six tiny per-block DMAs
                # serialized ~3us of HWDGE overhead right where hm1's
                # DoubleRow needed the data. xte8lo on the Act queue to
                # halve the chain.
                if GP8:
                    ecols = [
                        (c0, ln) for (ee, c0, ln) in blocks if ee == e
                    ]
                    ec0 = ecols[0][0]
                    ec1 = ecols[-1][0] + ecols[-1][1]
                    nc.sync.dma_start(
                        xte8[:, :, ec0:ec1], xtr8_d[:, :, ec0:ec1]
                    )
                    if CORR8:
                        nc.scalar.dma_start(
                            xte8lo[:, :, ec0:ec1], xtr8lo_d[:, :, ec0:ec1]
                        )
                wp_sb = wpp.tile([128, KD, MG, 128], BF16, tag="wp")
                nc.sync.dma_start(wp_sb[:], wpr_d[e])
                wp_tiles[e] = wp_sb

            for e in range(E):
                eblocks = [b for b in blocks if b[0] == e]
                # gate/value matmuls + silu-mult into ht
                for hm in range(MG):
                    wg, wl, wg8, wg8b = wv_tiles[(e, hm)]
                    fp8_here = wg8 is not None
                    fp8v_here = CORR8 and hm >= MG - 2 and e > 0
                    if fp8v_here:
                        wl8p, wl8bp = wv8_tiles[e]
                        wl8 = wl8p[:, hm - (MG - 2), :, :]
                        wl8b = wl8bp[:, hm - (MG - 2), :, :]
                    for (_, c0, ln) in eblocks:
                        psg = pg.tile([128, BLK], F32, tag="g")
                        k0 = 2 * GP8 if fp8_here else 0
                        for k in range(k0, KD):
                            nc.tensor.matmul(
                                psg[:, :ln],
                                lhsT=wg[:, k, :],
                                rhs=xte[:, k, c0 : c0 + ln],
                                start=(k == k0),
                                stop=(k == KD - 1 and not fp8_here),
                                skip_group_check=True,
                            )
                        if fp8_here:
                            # leading feature pairs in fp8 DoubleRow (256-deep
                            # contraction at 0.5 cycles/output-row), last in
                            # the group so bf16 work never waits on fp8 inputs
                            nc.tensor.matmul(
                                psg[:, :ln],
                                lhsT=wg8[:, :, :],
                                rhs=xte8[:, :, c0 : c0 + ln],
                                start=False,
                                stop=not CORR8,
                                perf_mode=PM.DoubleRow,
                                skip_group_check=True,
                            )
                            if CORR8:
                                # e5m2 correction: accumulates w*x_lo, exactly
                                # cancelling the main DR's x-quantization term
                                nc.tensor.matmul(
                                    psg[:, :ln],
                                    lhsT=wg8b[:, :, :],
                                    rhs=xte8lo[:, :, c0 : c0 + ln],
                                    start=False,
                                    stop=True,
                                    perf_mode=PM.DoubleRow,
                                    skip_group_check=True,
                                )
                        psv = pv.tile([128, BLK], F32, tag="v")
                        kv0 = 2 * GP8 if fp8v_here else 0
                        for k in range(kv0, KD):
                            nc.tensor.matmul(
                                psv[:, :ln],
                                lhsT=wl[:, k, :],
                                rhs=xte[:, k, c0 : c0 + ln],
                                start=(k == kv0),
                                stop=(k == KD - 1 and not fp8v_here),
                                skip_group_check=True,
                            )
                        if fp8v_here:
                            nc.tensor.matmul(
                                psv[:, :ln],
                                lhsT=wl8,
                                rhs=xte8[:, :, c0 : c0 + ln],
                                start=False,
                                stop=False,
                                perf_mode=PM.DoubleRow,
                                skip_group_check=True,
                            )
                            nc.tensor.matmul(
                                psv[:, :ln],
                                lhsT=wl8b,
                                rhs=xte8lo[:, :, c0 : c0 + ln],
                                start=False,
                                stop=True,
                                perf_mode=PM.DoubleRow,
                                skip_group_check=True,
                            )
                        sact = actp.tile([128, BLK], F32, tag="s")
                        nc.scalar.activation(sact[:, :ln], psg[:, :ln], AF.Silu)
                        nc.vector.tensor_tensor(
                            out=ht[:, hm, c0 : c0 + ln],
                            in0=sact[:, :ln],
                            in1=psv[:, :ln],
                            op=OP.mult,
                        )
                # proj: per token block, all 8 d-tiles, one output DMA
                wp_sb = wp_tiles[e]
                for (_, c0, ln) in eblocks:
                    ysb = outp.tile([128, KD, BLK], BF16, tag="y")
                    is_last = (e, c0, ln) == blocks[-1]
                    for d in range(KD):
                        psy = py.tile([128, BLK], F32, tag="py")
                        for k in range(MG):
                            nc.tensor.matmul(
                                psy[:, :ln],
                                lhsT=wp_sb[:, d, k, :],
                                rhs=ht[:, k, c0 : c0 + ln],
                                start=(k == 0),
                                stop=(k == MG - 1),
                            )
                        if is_last and d % 2 == 1 and d != KD - 1:
                            # final block: alternate copies onto the idle Act
                            # engine so the exit chain isn't DVE-serialized
                            nc.scalar.activation(
                                ysb[:, d, :ln], psy[:, :ln], AF.Copy
                            )
                        else:
                            nc.vector.tensor_copy(ysb[:, d, :ln], psy[:, :ln])
                        if is_last and d == KD - 2:
                            # drain d0..6 early so only d7's copy + a tiny
                            # DMA sit on the critical tail
                            nc.scalar.dma_start(
                                yt_d[:, : KD - 1, c0 : c0 + ln],
                                ysb[:, : KD - 1, :ln],
                            )
                    if is_last:
                        nc.sync.dma_start(
                            yt_d[:, KD - 1 :, c0 : c0 + ln],
                            ysb[:, KD - 1 :, :ln],
                        )
                    else:
                        nc.scalar.dma_start(
                            yt_d[:, :, c0 : c0 + ln], ysb[:, :, :ln]
                        )

    nc.compile()
    return nc


_NC = None
_NC_COUNTS = None


def _route(x, w_router):
    """Host router: f64 logits argmax (exactly matches the f32 reference
    argmax for any non-degenerate top-2 gap)."""
    x2 = np.asarray(x, dtype=np.float64).reshape(T, D)
    logits = x2 @ np.asarray(w_router, dtype=np.float64)
    eidx = np.argmax(logits, axis=1)
    counts = np.bincount(eidx, minlength=E)
    order = np.argsort(eidx, kind="stable")
    return eidx, counts, order


def _get_nc(counts=DEFAULT_COUNTS):
    global _NC, _NC_COUNTS
    counts = tuple(int(c) for c in counts)
    if _NC is None or _NC_COUNTS != counts:
        _NC = _build(counts)
        _NC_COUNTS = counts
    return _NC


def make_in_maps(x, w_v, w_proj, order):
    x2 = np.asarray(x, dtype=np.float32).reshape(T, D)
    wv = np.asarray(w_v, dtype=np.float32)
    wp = np.asarray(w_proj, dtype=np.float32)

    # compact transposed x, bf16: xtr[p, k, t] = x[order[t], k*128+p]
    xT = np.ascontiguousarray(x2[order].T)  # [D, T]
    xtr = np.ascontiguousarray(
        xT.reshape(KD, 128, T).transpose(1, 0, 2).astype(BF16NP)
    )
    if GP8:
        # fp8 copy of the leading GP8*256 features, pre-scaled by 1/S8,
        # DoubleRow slot-major: xtr8[p, 2j+i, t] = x[t, j*256+i*128+p]/S8
        xh8 = (xT[: GP8 * 256] / S8).astype(F8NP)
        xtr8 = np.ascontiguousarray(
            xh8.reshape(GP8 * 2, 128, T).transpose(1, 0, 2)
        )
        if CORR8:
            # residual x_lo = x - dequant(x8): encoded e5m2 at the same
            # 1/S8 scale so (S8*w)*(x_lo/S8) accumulates at true scale
            xlo = xT[: GP8 * 256] - xh8.astype(np.float32) * S8
            xtr8lo = np.ascontiguousarray(
                (xlo / S8)
                .astype(F8E5NP)
                .reshape(GP8 * 2, 128, T)
                .transpose(1, 0, 2)
            )

    in_maps = []
    for c in range(8):
        h0 = c * HS
        wvr_e = []
        wpr_e = []
        wvg8_e = []
        wvv8_e = []
        for e in range(E):
            gate = wv[e][:, h0 : h0 + HS]                   # [D, HS]
            val = wv[e][:, H + h0 : H + h0 + HS]            # [D, HS]
            wv_my = np.concatenate([gate, val], axis=1)     # [D, 2*HS]
            # wvr[m, p, k, c2] = wv_my[k*128+p, m*128+c2]
            wvr_e.append(
                wv_my.reshape(KD, 128, 2 * MG, 128).transpose(2, 1, 0, 3)
            )
            wp_my = wp[e][h0 : h0 + HS, :]                  # [HS, D]
            # wpr[p, d, k, c2] = wp_my[k*128+p, d*128+c2]
            wpr_e.append(
                wp_my.reshape(MG, 128, KD, 128).transpose(1, 2, 0, 3)
            )
            if GP8:
                # wvg8[hm][p, 2j+i, m] = gate[j*256+i*128+p, hm*128+m]*S8
                g8 = (gate[: GP8 * 256] * S8).reshape(GP8 * 2, 128, MG, 128)
                wvg8_e.append(g8.transpose(2, 1, 0, 3))  # [MG, 128, 2G, 128]
                v8 = (val[: GP8 * 256] * S8).reshape(GP8 * 2, 128, MG, 128)
                # hm 2-3 only, packed [128, 2, 2G, 128] for one DMA/expert
                wvv8_e.append(
                    v8.transpose(2, 1, 0, 3)[MG - 2 :].transpose(1, 0, 2, 3)
                )
        wvr = np.ascontiguousarray(np.concatenate(wvr_e, axis=0).astype(BF16NP))
        wpr = np.ascontiguousarray(np.stack(wpr_e, axis=0).astype(BF16NP))
        im = {"xtr": xtr, "wvr": wvr, "wpr": wpr}
        if GP8:
            wvg8 = np.concatenate(wvg8_e, axis=0)
            im["xtr8"] = xtr8
            im["wvg8"] = np.ascontiguousarray(wvg8.astype(F8NP))
            if CORR8:
                wvv8 = np.stack(wvv8_e, axis=0)
                im["xtr8lo"] = xtr8lo
                im["wvg8b"] = np.ascontiguousarray(wvg8.astype(F8E5NP))
                im["wvv8"] = np.ascontiguousarray(wvv8.astype(F8NP))
                im["wvv8b"] = np.ascontiguousarray(wvv8.astype(F8E5NP))
        in_maps.append(im)
    return in_maps


def combine(results, order):
    """Sum the 8 hidden-slice partial outputs and inverse-permute."""
    ysum = np.zeros((128, KD, T), dtype=np.float32)
    for r in results:
        ysum += np.asarray(r["yt"]).astype(np.float32)
    yT = ysum.transpose(1, 0, 2).reshape(D, T)  # [D, T] compact order
    out = np.empty((T, D), dtype=np.float32)
    out[order] = yT.T
    return out.reshape(2, 2048, D)


def kernel(x, w_router, w_v, w_proj):
    eidx, counts, order = _route(x, w_router)
    nc = _get_nc(counts)
    in_maps = make_in_maps(x, w_v, w_proj, order)
    res = run_bass_kernel_spmd(nc, in_maps, core_ids=list(range(8)), trace=False)
    return combine(res.results, order)


if __name__ == "__main__":
    sys.path.insert(0, "/root/problem")
    import reference

    ins = {k: np.asarray(v) for k, v in reference.setup_inputs().items()}
    got = kernel(**ins)
    exp = np.asarray(reference.reference(**ins))
    err = np.abs(got - exp)
    denom = np.abs(exp).max()
    print("max abs err:", err.max(), "rel:", err.max() / denom)
